# revision 33
# baseline (speedup 1.0000x reference)
# Multi-head attention (N=4, L=2048, D=1024, H=16, DK=64) on 8 NeuronCores.
#
# Sharding: batch x head-half tensor-parallel. Core c = (n, hh) computes the
# full 2048-q attention of batch n for heads [hh*8, hh*8+8), then the partial
# output projection over its 512 e-dims (WO row-sharded). The host sums the
# two partials per batch and adds the bias.
#
# Per-core pipeline (measured ~395us wall; PE ~346us busy, ScalarE ~297us,
# DVE ~253us; ScalarE exp paces the steady-state passes at ~18.6us/pass):
#   PE     : V/K/Q projections, S^T = KiT^T QiT (contract 64), PV (contract
#            128, M=128 with replicated ones blocks for row sums), partial
#            out-proj. All matmuls bf16, N=512 moving.
#   ScalarE: exp(S/8) [128,1024] psum->sbuf bf16; rowsum reciprocal as
#            exp(-ln(x)) (both functions pinned to one activation table via
#            _patch_act_tables -- DVE reciprocal is 3.4us per call, the
#            custom-DVE approx NaNs on HW from PSUM, and ScalarE Reciprocal
#            is blocked by bass).
#   DVE    : mask multiply (broadcast bf16, 2x mode), head evacuations
#            (single-input copies may cross partition bases; TensorTensor
#            may not), normalize multiply, projection/out evacuations.
#   DMA    : all inputs host-pre-tiled bf16 so every transfer is a large
#            per-partition-contiguous run (4-16KB packets, ~10k packets vs
#            45k strided); issue order feeds the projection-era critical
#            path (wv, v0, wk, k0 first; q/mask deferred) so PE starts at
#            ~14us instead of ~43us.
#
# Row-sum trick: the PV stationary matrix per head pair is laid out
# [onesA(64) | ViA(64) | ViB(64) | onesB(64)] (256 cols). Head A uses cols
# [0:128) so its PV psum has rows 0-63 = the row sum replicated across 64
# partitions and rows 64-127 = head data; head B uses cols [128:256) giving
# the mirror image. Row-sum reciprocals then land partition-ALIGNED for the
# normalize multiply, and normalization costs zero PE cycles and no DRAM
# round trip (PV matmul cost is the moving dim only; M=65 vs 128 is free).
# QiT/headiT are per-head-pair tiles (dependency tracking is tile-granular).
# PSUM: st 2x2 banks + pv 2 + proj/out 2 = 8 (full).
# bf16 K/Q/weight inputs: rel err 0.0121 vs 2e-2 budget (f32r inputs gave
# 0.0091 but cost ~25us in DMA/SBUF pressure).
import sys

sys.path.insert(0, "/opt/trn_rl_repo")

import collections
from contextlib import ExitStack

import numpy as np
import ml_dtypes

N, QLEN, KLEN, DMODEL, NHEAD, DK = 4, 2048, 2048, 1024, 16, 64
NCORES = 8
P = 128
HPC = NHEAD // 2  # 8 heads per core
EH = HPC * DK  # 512 e-dims per core
EO = EH // P  # 4 e-tiles (= head pairs)
DO = DMODEL // P  # 8 d-tiles
KO = KLEN // P  # 16 k-tiles
NQQ = 4  # attention q-blocks
QQ = QLEN // NQQ  # 512 q per block
SKEW = 4  # PV trails S/exp/mask by this many k-tiles
VB = 256  # Vi cols per head pair: [onesA(64) | ViA(64) | ViB(64) | onesB(64)]
QT = QLEN // P  # 16 q-tiles of 128 for the output

_prog_cache = {}


def _patch_act_tables():
    # The act-table placement pass serves each activation from the first
    # table containing its function, so a kernel using Exp and Ln ping-pongs
    # between 'exp_and_others' and 'natural_log' (1283ns per reload, ~47
    # reloads here). Both live in 'natural_log_exp_and_others'; hide Exp/Ln
    # from every other table (preserving table order/indices, which are the
    # act_func_set_id the runtime loads) so the pass settles on the shared
    # one. The real table loaded at runtime is unmodified and serves both.
    import concourse.bacc as bacc_mod
    import concourse.hw_specs as hw_specs

    if getattr(bacc_mod, "_act_tables_patched", False):
        return
    orig = hw_specs.get_activation_tables

    def patched(arch):
        out = {}
        for name, s in orig(arch).items():
            if name != "natural_log_exp_and_others":
                s = {
                    f
                    for f in s
                    if str(f).rsplit(".", 1)[-1] not in ("Exp", "Ln")
                }
            out[name] = s
        return out

    for mod in (bacc_mod, hw_specs):
        if hasattr(mod, "get_activation_tables"):
            mod.get_activation_tables = patched
    bacc_mod._act_tables_patched = True


def _build_program():
    import concourse.tile as tile
    from concourse import bacc, mybir

    _patch_act_tables()

    f32 = mybir.dt.float32
    f32r = mybir.dt.float32r
    bf16 = mybir.dt.bfloat16
    Exp = mybir.ActivationFunctionType.Exp
    Ln = mybir.ActivationFunctionType.Ln

    nc = bacc.Bacc("TRN2", target_bir_lowering=False, debug=False)

    # Host-pre-tiled inputs: each DMA below is contiguous per partition.
    qT_t = nc.dram_tensor("qT", (NQQ, P, DO, QQ), bf16, kind="ExternalInput").ap()
    kT_t = nc.dram_tensor("kT", (4, P, DO, 512), bf16, kind="ExternalInput").ap()
    vT_t = nc.dram_tensor("vT", (8, P, DO, 256), bf16, kind="ExternalInput").ap()
    maskT_t = nc.dram_tensor(
        "maskT", (NQQ, 2, P, KO // 2, QQ), bf16, kind="ExternalInput"
    ).ap()
    wq_t = nc.dram_tensor("wq", (EO, P, DO, P), bf16, kind="ExternalInput").ap()
    wk_t = nc.dram_tensor("wk", (P, EO, DO, P), bf16, kind="ExternalInput").ap()
    wv_t = nc.dram_tensor("wv", (P, DO, EH), bf16, kind="ExternalInput").ap()
    wo_t = nc.dram_tensor("wo", (P, EO, DMODEL), bf16, kind="ExternalInput").ap()
    out_t = nc.dram_tensor("out", (QT, 2, P, 512), bf16, kind="ExternalOutput").ap()

    with tile.TileContext(nc) as tc, ExitStack() as top:
        res = top.enter_context(tc.tile_pool(name="res", bufs=1))
        # per-head-pair tiles: dependency tracking is tile-granular, so a
        # shared tile would serialize S matmuls behind unrelated q_proj
        # copies and out-proj behind the last pass's normalize
        QiT_s = [
            res.tile([P, QLEN], bf16, name=f"QiT{e}", tag=f"QiT{e}")
            for e in range(EO)
        ]
        KiT_s = res.tile([P, EO, KLEN], bf16)
        Vi_s = res.tile([P, KO, EO * VB], bf16)  # per pair [ViA|ones|ViB]
        headiT_s = [
            res.tile([P, QLEN], bf16, name=f"headiT{e}", tag=f"headiT{e}")
            for e in range(EO)
        ]
        wo_s = res.tile([P, EO, DMODEL], bf16)
        wq_s = res.tile([P, EO, DO, P], bf16)

        gps = top.enter_context(tc.tile_pool(name="gpsum", bufs=2, space="PSUM"))
        sps = top.enter_context(tc.tile_pool(name="spsum", bufs=2, space="PSUM"))
        pvs = top.enter_context(tc.tile_pool(name="pvsum", bufs=1, space="PSUM"))
        ptp = top.enter_context(tc.tile_pool(name="ptile", bufs=SKEW + 3))
        mkp = top.enter_context(tc.tile_pool(name="mask", bufs=3))
        qsp = top.enter_context(tc.tile_pool(name="qstripe", bufs=2))
        nrm = top.enter_context(tc.tile_pool(name="norm", bufs=2))
        lnp = top.enter_context(tc.tile_pool(name="lnp", bufs=2))
        rtp = top.enter_context(tc.tile_pool(name="rtp", bufs=2))

        mask_tiles = {}
        q_stripes = {}
        q_psum = {}
        q_done = set()
        filler = collections.deque()

        def drain(n):
            for _ in range(n):
                if not filler:
                    return
                item = filler.popleft()
                if callable(item):
                    item()
                else:  # generator: run one step, put its next step up front
                    try:
                        next(item)
                        filler.appendleft(item)
                    except StopIteration:
                        pass

        def load_mask(qq):
            mks = []
            for h in range(2):
                mk = mkp.tile(
                    [P, KO // 2, QQ], bf16, tag="mask", name=f"mask{qq}_{h}"
                )
                nc.sync.dma_start(mk[:], maskT_t[qq, h])
                mks.append(mk)
            mask_tiles[qq] = mks

        def load_qstripe(qq):
            qs = qsp.tile([P, DO, QQ], bf16, tag="qT", name=f"qT{qq}")
            nc.sync.dma_start(qs[:], qT_t[qq])
            q_stripes[qq] = qs

        def q_proj(qq, eo, halves=(0, 1)):
            if 0 in halves:
                self_pt = gps.tile([P, 512], f32, tag="gps", name=f"psq{qq}_{eo}")
                q_psum[(qq, eo)] = self_pt
            pt = q_psum[(qq, eo)]
            qs = q_stripes[qq]
            for h in halves:
                for do in range(h * DO // 2, (h + 1) * DO // 2):
                    nc.tensor.matmul(
                        pt[:],
                        lhsT=wq_s[:, eo, do],
                        rhs=qs[:, do],
                        start=(do == 0),
                        stop=(do == DO - 1),
                    )
            if 1 in halves:
                nc.vector.tensor_copy(
                    out=QiT_s[eo][:, qq * QQ : (qq + 1) * QQ], in_=pt[:]
                )
                del q_psum[(qq, eo)]
                q_done.add((qq, eo))

        class AttnPass:
            def __init__(self, qq, hp):
                self.qq, self.hp = qq, hp
                self.mk = mask_tiles[qq]
                self.pv = None
                self.ptq = {}
                self.next_ko = 0

            def emit_pv(self, ko):
                if self.pv is None:
                    # allocated lazily so the pool sees this AFTER the
                    # previous pass's trailing reads (cross-pass pipelining)
                    self.pv = [
                        pvs.tile(
                            [P, QQ], f32, tag=f"pv{i}",
                            name=f"pv{i}_{self.qq}_{self.hp}",
                        )
                        for i in range(2)
                    ]
                pt = self.ptq.pop(ko)
                base = self.hp * VB
                for i in range(2):
                    nc.tensor.matmul(
                        self.pv[i][:],
                        lhsT=Vi_s[:, ko, base + 128 * i : base + 128 * i + 128],
                        rhs=pt[:, i * QQ : (i + 1) * QQ],
                        start=(ko == 0),
                        stop=(ko == KO - 1),
                        skip_group_check=True,
                    )

            def steps(self, ko_end, do_drain=True):
                qq, hp = self.qq, self.hp
                for ko in range(self.next_ko, ko_end):
                    if ko >= SKEW:
                        self.emit_pv(ko - SKEW)
                    st = sps.tile([P, 2 * QQ], f32, tag="st", name=f"st{qq}_{hp}_{ko}")
                    for i in range(2):
                        p0 = 64 * i
                        nc.tensor.matmul(
                            st[:, i * QQ : (i + 1) * QQ],
                            lhsT=KiT_s[p0 : p0 + 64, hp, ko * P : (ko + 1) * P],
                            rhs=QiT_s[hp][p0 : p0 + 64, qq * QQ : (qq + 1) * QQ],
                            start=True,
                            stop=True,
                        )
                    pt = ptp.tile([P, 2 * QQ], bf16, tag="pt", name=f"pt{qq}_{hp}_{ko}")
                    nc.scalar.activation(out=pt[:], in_=st[:], func=Exp, scale=0.125)
                    # mask multiply stays on DVE only: GpSimd tensor ops are
                    # ~3x slower than modeled with jitter that starves the pt
                    # queue
                    mkt = self.mk[ko // 8]
                    nc.vector.tensor_mul(
                        out=pt[:].rearrange("p (i q) -> p i q", i=2),
                        in0=pt[:].rearrange("p (i q) -> p i q", i=2),
                        in1=mkt[:, ko % 8, None, :].to_broadcast([P, 2, QQ]),
                    )
                    self.ptq[ko] = pt
                    if do_drain and ko % 2 == 1:
                        drain(1)
                self.next_ko = ko_end

            def finish(self):
                qq, hp = self.qq, self.hp
                for ko in range(KO - SKEW, KO):
                    self.emit_pv(ko)
                qsl = slice(qq * QQ, (qq + 1) * QQ)
                # pv0: rows 0-63 rowsum A (replicated), 64-127 head A data;
                # pv1: rows 0-63 head B data, 64-127 rowsum B.
                # Rowsum reciprocal as exp(-ln(x)) on ScalarE: Ln and Exp
                # share an activation table (no table-switch cost), DVE
                # divide is ~3.4us per call here, and ScalarE has slack. The
                # rowsum halves land partition-aligned by the Vi layout; head
                # evacuations are single-input copies so they may cross
                # partition bases (TensorTensor may not).
                lnt = lnp.tile([P, QQ], f32, tag="ln", name=f"ln{qq}_{hp}")
                rt = rtp.tile([P, QQ], bf16, tag="rt", name=f"rt{qq}_{hp}")
                nc.vector.tensor_copy(
                    out=headiT_s[hp][0:64, qsl], in_=self.pv[0][64:128, :]
                )
                nc.scalar.activation(
                    out=lnt[0:64, :], in_=self.pv[0][0:64, :], func=Ln, scale=1.0
                )
                nc.vector.tensor_copy(
                    out=headiT_s[hp][64:128, qsl], in_=self.pv[1][0:64, :]
                )
                nc.scalar.activation(
                    out=lnt[64:128, :], in_=self.pv[1][64:128, :], func=Ln, scale=1.0
                )
                nc.scalar.activation(out=rt[:], in_=lnt[:], func=Exp, scale=-1.0)
                nc.vector.tensor_mul(
                    out=headiT_s[hp][:, qsl],
                    in0=headiT_s[hp][:, qsl],
                    in1=rt[:],
                )

        # ---------- projection era: V + K stripes, pass (0,0) streamed ----
        p00 = None
        with ExitStack() as ph:
            wvp = ph.enter_context(tc.tile_pool(name="wvres", bufs=1))
            wkp = ph.enter_context(tc.tile_pool(name="wkp", bufs=1))
            vsp = ph.enter_context(tc.tile_pool(name="vstripe", bufs=3))
            ksp = ph.enter_context(tc.tile_pool(name="kstripe", bufs=2))

            vtiles = {}
            ktiles = {}

            def load_v(hs):
                vs = vsp.tile([P, DO, 256], bf16, tag="v", name=f"vT{hs}")
                nc.sync.dma_start(vs[:], vT_t[hs])
                vtiles[hs] = vs

            def load_k(s):
                ks = ksp.tile([P, DO, 512], bf16, tag="k", name=f"kT{s}")
                nc.sync.dma_start(ks[:], kT_t[s])
                ktiles[s] = ks

            # DMA issue order = arrival order; feed the V/K critical path
            # first so PE starts ~35us earlier than loading q/mask up front.
            wv_s = wvp.tile([P, DO, EH], bf16)
            nc.sync.dma_start(wv_s[:], wv_t[:])
            load_v(0)
            wk_s = wkp.tile([P, EO, DO, P], bf16)
            nc.sync.dma_start(wk_s[:], wk_t[:])
            load_k(0)
            nc.sync.dma_start(wq_s[:, 0], wq_t[0])
            load_qstripe(0)
            load_v(1)
            load_k(1)
            load_mask(0)
            load_v(2)
            # ones blocks for the PV rowsum trick
            vi_all = Vi_s[:].rearrange("p k (pr c) -> p k pr c", c=VB)
            nc.vector.memset(vi_all[:, :, :, 0:64], 1.0)
            nc.vector.memset(vi_all[:, :, :, 192:256], 1.0)

            for s in range(4):
                for half in range(2):
                    hs = 2 * s + half
                    vs = vtiles.pop(hs)
                    for t in range(2):
                        ko = hs * 2 + t
                        pt = gps.tile([P, 512], f32, tag="gps", name=f"psv{ko}")
                        for do in range(DO):
                            nc.tensor.matmul(
                                pt[:],
                                lhsT=vs[:, do, t * P : (t + 1) * P],
                                rhs=wv_s[:, do],
                                start=(do == 0),
                                stop=(do == DO - 1),
                            )
                        vi_ko = Vi_s[:, ko].rearrange("p (pr c) -> p pr c", c=VB)
                        nc.vector.tensor_copy(
                            out=vi_ko[:, :, 64:192],
                            in_=pt[:].rearrange("p (pr e) -> p pr e", e=128),
                        )
                    if hs + 3 <= 7:
                        load_v(hs + 3)
                ks = ktiles.pop(s)
                for eo in range(EO):
                    pt = gps.tile([P, 512], f32, tag="gps", name=f"psk{s}_{eo}")
                    for do in range(DO):
                        nc.tensor.matmul(
                            pt[:],
                            lhsT=wk_s[:, eo, do],
                            rhs=ks[:, do],
                            start=(do == 0),
                            stop=(do == DO - 1),
                        )
                    nc.vector.tensor_copy(
                        out=KiT_s[:, eo, s * 512 : (s + 1) * 512], in_=pt[:]
                    )
                # prefetches + deferred loads, then stream pass (0,0):
                # its 16 exps retire on the otherwise-idle ScalarE during
                # the projection era (ScalarE paces the steady state)
                if s == 0:
                    load_k(2)
                    q_proj(0, 0)
                    p00 = AttnPass(0, 0)
                    p00.steps(4, do_drain=False)
                elif s == 1:
                    load_k(3)
                    p00.steps(8, do_drain=False)
                elif s == 2:
                    for eo in range(1, EO):
                        nc.sync.dma_start(wq_s[:, eo], wq_t[eo])
                    nc.sync.dma_start(wo_s[:], wo_t[:])
                    p00.steps(12, do_drain=False)
                elif s == 3:
                    p00.steps(KO, do_drain=False)

        def out_proj(qq):
            for t in range(QQ // P):  # 4 q-tiles of 128
                qt = qq * (QQ // P) + t
                for dc in range(2):
                    pt = gps.tile([P, 512], f32, tag="gps", name=f"pso{qt}_{dc}")
                    # split the accumulation so the eo<3 matmuls don't
                    # inherit the last head's normalize dependency (waits
                    # hoist to the group head); lets out-proj overlap the
                    # final pass's normalize chain
                    for eo in range(EO):
                        nc.tensor.matmul(
                            pt[:],
                            lhsT=headiT_s[eo][:, qt * P : (qt + 1) * P],
                            rhs=wo_s[:, eo, dc * 512 : (dc + 1) * 512],
                            start=(eo == 0),
                            stop=(eo >= EO - 2),
                            skip_group_check=(eo == EO - 1),
                        )
                    ot = nrm.tile([P, 512], bf16, tag="ot", name=f"o{qt}_{dc}")
                    nc.vector.tensor_copy(out=ot[:], in_=pt[:])
                    nc.sync.dma_start(out_t[qt, dc], ot[:])
                    yield

        for eo in range(1, EO):
            filler.append(lambda eo=eo: q_proj(0, eo, (0,)))
            filler.append(lambda eo=eo: q_proj(0, eo, (1,)))

        def queue_setup(qq):
            filler.append(lambda: (load_mask(qq), load_qstripe(qq)))
            for eo in range(EO):
                filler.append(lambda eo=eo: q_proj(qq, eo, (0,)))
                filler.append(lambda eo=eo: q_proj(qq, eo, (1,)))

        # ---------- attention passes (software-pipelined across passes) ---
        def notify_finished(p):
            if p.hp == EO - 1:
                filler.append(out_proj(p.qq))

        prev = None
        for qq in range(NQQ):
            if qq + 1 < NQQ:
                queue_setup(qq + 1)
            for hp in range(EO):
                if qq == 0 and hp == 0:
                    p00.steps(KO, do_drain=False)
                    prev = p00
                    continue
                while (qq, hp) not in q_done or qq not in mask_tiles:
                    drain(1)
                ap = AttnPass(qq, hp)
                ap.steps(SKEW, do_drain=False)
                if prev is not None:
                    prev.finish()
                    notify_finished(prev)
                    # stuff the pass boundary with filler so PE covers the
                    # DVE pv-drain (WAR on the recycled pv psum banks)
                    drain(3)
                ap.steps(KO)
                prev = ap
        prev.finish()
        notify_finished(prev)
        while filler:
            drain(1)

    nc.compile()
    return nc


def get_program():
    if "nc" not in _prog_cache:
        _prog_cache["nc"] = _build_program()
    return _prog_cache["nc"]


def make_in_maps(K, Q, V, mask, WQ, WK, WV, WO_w, WO_b):
    bf = ml_dtypes.bfloat16
    K = np.asarray(K, dtype=np.float32)
    Q = np.asarray(Q, dtype=np.float32)
    V = np.asarray(V, dtype=np.float32)
    mask = np.asarray(mask)
    WQ = np.asarray(WQ, dtype=np.float32)
    WK = np.asarray(WK, dtype=np.float32)
    WV = np.asarray(WV, dtype=np.float32)
    woT = np.asarray(WO_w, dtype=np.float32).T  # (E, DMODEL)

    def tile_dq(xT, stripes, width):
        # (D, L) -> (stripes, P, DO, width), contiguous per partition
        return np.ascontiguousarray(
            xT.reshape(DO, P, stripes, width).transpose(2, 1, 0, 3)
        )

    qT_b = [tile_dq(Q[n].T.astype(bf), NQQ, QQ) for n in range(N)]
    kT_b = [tile_dq(K[n].T.astype(bf), 4, 512) for n in range(N)]
    vT_b = [tile_dq(V[n].T.astype(bf), 8, 256) for n in range(N)]
    maskT_b = [
        np.ascontiguousarray(
            mask[n, 0]
            .T.astype(bf)
            .reshape(2, KO // 2, P, NQQ, QQ)
            .transpose(3, 0, 2, 1, 4)
        )
        for n in range(N)
    ]

    in_maps = []
    for c in range(NCORES):
        n, hh = c // 2, c % 2
        hs = slice(hh * HPC, (hh + 1) * HPC)
        # head-concat weight slices: (HPC, D, DK) -> (D, HPC*DK)
        wq_h = np.ascontiguousarray(WQ[hs].transpose(1, 0, 2).reshape(DMODEL, EH))
        wk_h = np.ascontiguousarray(WK[hs].transpose(1, 0, 2).reshape(DMODEL, EH))
        wv_h = WV[hs].transpose(1, 0, 2).reshape(DMODEL, EH)
        wo_h = woT[hh * EH : (hh + 1) * EH, :]
        in_maps.append(
            {
                "qT": qT_b[n],
                "kT": kT_b[n],
                "vT": vT_b[n],
                "maskT": maskT_b[n],
                "wq": np.ascontiguousarray(
                    wq_h.reshape(DO, P, EO, P).transpose(2, 1, 0, 3).astype(bf)
                ),
                "wk": np.ascontiguousarray(
                    wk_h.reshape(DO, P, EO, P).transpose(1, 2, 0, 3).astype(bf)
                ),
                "wv": np.ascontiguousarray(
                    wv_h.reshape(DO, P, EH).transpose(1, 0, 2).astype(bf)
                ),
                "wo": np.ascontiguousarray(
                    wo_h.reshape(EO, P, DMODEL).transpose(1, 0, 2).astype(bf)
                ),
            }
        )
    return in_maps


def kernel(K, Q, V, mask, WQ, WK, WV, WO_w, WO_b):
    from concourse import bass_utils

    nc = get_program()
    in_maps = make_in_maps(K, Q, V, mask, WQ, WK, WV, WO_w, WO_b)
    res = bass_utils.run_bass_kernel_spmd(
        nc, in_maps, core_ids=list(range(NCORES)), trace=False
    )
    bias = np.asarray(WO_b, dtype=np.float32).reshape(1, DMODEL)
    out = np.empty((N, QLEN, DMODEL), dtype=np.float32)
    for n in range(N):
        o0 = res.results[2 * n]["out"].astype(np.float32)
        o1 = res.results[2 * n + 1]["out"].astype(np.float32)
        full = (o0 + o1).transpose(0, 2, 1, 3).reshape(QLEN, DMODEL)
        out[n] = full + bias
    return out


# revision 35
# speedup vs baseline: 1.0069x; 1.0069x over previous
# Multi-head attention (N=4, L=2048, D=1024, H=16, DK=64) on 8 NeuronCores.
#
# Sharding: batch x head-half tensor-parallel. Core c = (n, hh) computes the
# full 2048-q attention of batch n for heads [hh*8, hh*8+8), then the partial
# output projection over its 512 e-dims (WO row-sharded). The host sums the
# two partials per batch and adds the bias.
#
# Per-core pipeline (measured ~395us wall; PE ~346us busy, ScalarE ~297us,
# DVE ~253us; ScalarE exp paces the steady-state passes at ~18.6us/pass):
#   PE     : V/K/Q projections, S^T = KiT^T QiT (contract 64), PV (contract
#            128, M=128 with replicated ones blocks for row sums), partial
#            out-proj. All matmuls bf16, N=512 moving.
#   ScalarE: exp(S/8) [128,1024] psum->sbuf bf16; rowsum reciprocal as
#            exp(-ln(x)) (both functions pinned to one activation table via
#            _patch_act_tables -- DVE reciprocal is 3.4us per call, the
#            custom-DVE approx NaNs on HW from PSUM, and ScalarE Reciprocal
#            is blocked by bass).
#   DVE    : mask multiply (broadcast bf16, 2x mode), head evacuations
#            (single-input copies may cross partition bases; TensorTensor
#            may not), normalize multiply, projection/out evacuations.
#   DMA    : all inputs host-pre-tiled bf16 so every transfer is a large
#            per-partition-contiguous run (4-16KB packets, ~10k packets vs
#            45k strided); issue order feeds the projection-era critical
#            path (wv, v0, wk, k0 first; q/mask deferred) so PE starts at
#            ~14us instead of ~43us.
#
# Row-sum trick: the PV stationary matrix per head pair is laid out
# [onesA(64) | ViA(64) | ViB(64) | onesB(64)] (256 cols). Head A uses cols
# [0:128) so its PV psum has rows 0-63 = the row sum replicated across 64
# partitions and rows 64-127 = head data; head B uses cols [128:256) giving
# the mirror image. Row-sum reciprocals then land partition-ALIGNED for the
# normalize multiply, and normalization costs zero PE cycles and no DRAM
# round trip (PV matmul cost is the moving dim only; M=65 vs 128 is free).
# QiT/headiT are per-head-pair tiles (dependency tracking is tile-granular).
# PSUM: st 2x2 banks + pv 2 + proj/out 2 = 8 (full).
# bf16 K/Q/weight inputs: rel err 0.0121 vs 2e-2 budget (f32r inputs gave
# 0.0091 but cost ~25us in DMA/SBUF pressure).
import sys

sys.path.insert(0, "/opt/trn_rl_repo")

import collections
from contextlib import ExitStack

import numpy as np
import ml_dtypes

N, QLEN, KLEN, DMODEL, NHEAD, DK = 4, 2048, 2048, 1024, 16, 64
NCORES = 8
P = 128
HPC = NHEAD // 2  # 8 heads per core
EH = HPC * DK  # 512 e-dims per core
EO = EH // P  # 4 e-tiles (= head pairs)
DO = DMODEL // P  # 8 d-tiles
KO = KLEN // P  # 16 k-tiles
NQQ = 4  # attention q-blocks
QQ = QLEN // NQQ  # 512 q per block
SKEW = 4  # PV trails S/exp/mask by this many k-tiles
VB = 256  # Vi cols per head pair: [onesA(64) | ViA(64) | ViB(64) | onesB(64)]
QT = QLEN // P  # 16 q-tiles of 128 for the output

_prog_cache = {}


def _patch_act_tables():
    # The act-table placement pass serves each activation from the first
    # table containing its function, so a kernel using Exp and Ln ping-pongs
    # between 'exp_and_others' and 'natural_log' (1283ns per reload, ~47
    # reloads here). Both live in 'natural_log_exp_and_others'; hide Exp/Ln
    # from every other table (preserving table order/indices, which are the
    # act_func_set_id the runtime loads) so the pass settles on the shared
    # one. The real table loaded at runtime is unmodified and serves both.
    import concourse.bacc as bacc_mod
    import concourse.hw_specs as hw_specs

    if getattr(bacc_mod, "_act_tables_patched", False):
        return
    orig = hw_specs.get_activation_tables

    def patched(arch):
        out = {}
        for name, s in orig(arch).items():
            if name != "natural_log_exp_and_others":
                s = {
                    f
                    for f in s
                    if str(f).rsplit(".", 1)[-1] not in ("Exp", "Ln")
                }
            out[name] = s
        return out

    for mod in (bacc_mod, hw_specs):
        if hasattr(mod, "get_activation_tables"):
            mod.get_activation_tables = patched
    bacc_mod._act_tables_patched = True


def _build_program():
    import concourse.tile as tile
    from concourse import bacc, mybir

    _patch_act_tables()

    f32 = mybir.dt.float32
    f32r = mybir.dt.float32r
    bf16 = mybir.dt.bfloat16
    Exp = mybir.ActivationFunctionType.Exp
    Ln = mybir.ActivationFunctionType.Ln

    nc = bacc.Bacc("TRN2", target_bir_lowering=False, debug=False)

    # Host-pre-tiled inputs: each DMA below is contiguous per partition.
    qT_t = nc.dram_tensor("qT", (NQQ, P, DO, QQ), bf16, kind="ExternalInput").ap()
    kT_t = nc.dram_tensor("kT", (4, P, DO, 512), bf16, kind="ExternalInput").ap()
    vT_t = nc.dram_tensor("vT", (8, P, DO, 256), bf16, kind="ExternalInput").ap()
    maskT_t = nc.dram_tensor(
        "maskT", (NQQ, 2, P, KO // 2, QQ), bf16, kind="ExternalInput"
    ).ap()
    wq_t = nc.dram_tensor("wq", (EO, P, DO, P), bf16, kind="ExternalInput").ap()
    wk_t = nc.dram_tensor("wk", (P, EO, DO, P), bf16, kind="ExternalInput").ap()
    wv_t = nc.dram_tensor("wv", (P, DO, EH), bf16, kind="ExternalInput").ap()
    wo_t = nc.dram_tensor("wo", (P, EO, DMODEL), bf16, kind="ExternalInput").ap()
    out_t = nc.dram_tensor("out", (QT, 2, P, 512), bf16, kind="ExternalOutput").ap()
    # quad-A partial of the last q-block, out-projected early so only half
    # the out-proj remains after the final pass (host adds it in)
    out2_t = nc.dram_tensor("out2", (4, 2, P, 512), bf16, kind="ExternalOutput").ap()

    with tile.TileContext(nc) as tc, ExitStack() as top:
        res = top.enter_context(tc.tile_pool(name="res", bufs=1))
        # per-head-pair tiles: dependency tracking is tile-granular, so a
        # shared tile would serialize S matmuls behind unrelated q_proj
        # copies and out-proj behind the last pass's normalize
        QiT_s = [
            res.tile([P, QLEN], bf16, name=f"QiT{e}", tag=f"QiT{e}")
            for e in range(EO)
        ]
        KiT_s = res.tile([P, EO, KLEN], bf16)
        Vi_s = res.tile([P, KO, EO * VB], bf16)  # per pair [ViA|ones|ViB]
        headiT_s = [
            res.tile([P, QLEN], bf16, name=f"headiT{e}", tag=f"headiT{e}")
            for e in range(EO)
        ]
        wo_s = res.tile([P, EO, DMODEL], bf16)
        wq_s = res.tile([P, EO, DO, P], bf16)

        gps = top.enter_context(tc.tile_pool(name="gpsum", bufs=2, space="PSUM"))
        sps = top.enter_context(tc.tile_pool(name="spsum", bufs=2, space="PSUM"))
        pvs = top.enter_context(tc.tile_pool(name="pvsum", bufs=1, space="PSUM"))
        ptp = top.enter_context(tc.tile_pool(name="ptile", bufs=SKEW + 3))
        mkp = top.enter_context(tc.tile_pool(name="mask", bufs=3))
        qsp = top.enter_context(tc.tile_pool(name="qstripe", bufs=2))
        nrm = top.enter_context(tc.tile_pool(name="norm", bufs=2))
        lnp = top.enter_context(tc.tile_pool(name="lnp", bufs=2))
        rtp = top.enter_context(tc.tile_pool(name="rtp", bufs=2))

        mask_tiles = {}
        q_stripes = {}
        q_psum = {}
        q_done = set()
        filler = collections.deque()

        def drain(n):
            for _ in range(n):
                if not filler:
                    return
                item = filler.popleft()
                if callable(item):
                    item()
                else:  # generator: run one step, put its next step up front
                    try:
                        next(item)
                        filler.appendleft(item)
                    except StopIteration:
                        pass

        def load_mask(qq):
            mks = []
            for h in range(2):
                mk = mkp.tile(
                    [P, KO // 2, QQ], bf16, tag="mask", name=f"mask{qq}_{h}"
                )
                nc.sync.dma_start(mk[:], maskT_t[qq, h])
                mks.append(mk)
            mask_tiles[qq] = mks

        def load_qstripe(qq):
            qs = qsp.tile([P, DO, QQ], bf16, tag="qT", name=f"qT{qq}")
            nc.sync.dma_start(qs[:], qT_t[qq])
            q_stripes[qq] = qs

        def q_proj(qq, eo, halves=(0, 1)):
            if 0 in halves:
                self_pt = gps.tile([P, 512], f32, tag="gps", name=f"psq{qq}_{eo}")
                q_psum[(qq, eo)] = self_pt
            pt = q_psum[(qq, eo)]
            qs = q_stripes[qq]
            for h in halves:
                for do in range(h * DO // 2, (h + 1) * DO // 2):
                    nc.tensor.matmul(
                        pt[:],
                        lhsT=wq_s[:, eo, do],
                        rhs=qs[:, do],
                        start=(do == 0),
                        stop=(do == DO - 1),
                    )
            if 1 in halves:
                nc.vector.tensor_copy(
                    out=QiT_s[eo][:, qq * QQ : (qq + 1) * QQ], in_=pt[:]
                )
                del q_psum[(qq, eo)]
                q_done.add((qq, eo))

        class AttnPass:
            def __init__(self, qq, hp):
                self.qq, self.hp = qq, hp
                self.mk = mask_tiles[qq]
                self.pv = None
                self.ptq = {}
                self.next_ko = 0

            def emit_pv(self, ko):
                if self.pv is None:
                    # allocated lazily so the pool sees this AFTER the
                    # previous pass's trailing reads (cross-pass pipelining)
                    self.pv = [
                        pvs.tile(
                            [P, QQ], f32, tag=f"pv{i}",
                            name=f"pv{i}_{self.qq}_{self.hp}",
                        )
                        for i in range(2)
                    ]
                pt = self.ptq.pop(ko)
                base = self.hp * VB
                for i in range(2):
                    nc.tensor.matmul(
                        self.pv[i][:],
                        lhsT=Vi_s[:, ko, base + 128 * i : base + 128 * i + 128],
                        rhs=pt[:, i * QQ : (i + 1) * QQ],
                        start=(ko == 0),
                        stop=(ko == KO - 1),
                        skip_group_check=True,
                    )

            def steps(self, ko_end, do_drain=True):
                qq, hp = self.qq, self.hp
                for ko in range(self.next_ko, ko_end):
                    if ko >= SKEW:
                        self.emit_pv(ko - SKEW)
                    st = sps.tile([P, 2 * QQ], f32, tag="st", name=f"st{qq}_{hp}_{ko}")
                    for i in range(2):
                        p0 = 64 * i
                        nc.tensor.matmul(
                            st[:, i * QQ : (i + 1) * QQ],
                            lhsT=KiT_s[p0 : p0 + 64, hp, ko * P : (ko + 1) * P],
                            rhs=QiT_s[hp][p0 : p0 + 64, qq * QQ : (qq + 1) * QQ],
                            start=True,
                            stop=True,
                        )
                    pt = ptp.tile([P, 2 * QQ], bf16, tag="pt", name=f"pt{qq}_{hp}_{ko}")
                    nc.scalar.activation(out=pt[:], in_=st[:], func=Exp, scale=0.125)
                    # mask multiply stays on DVE only: GpSimd tensor ops are
                    # ~3x slower than modeled with jitter that starves the pt
                    # queue
                    mkt = self.mk[ko // 8]
                    nc.vector.tensor_mul(
                        out=pt[:].rearrange("p (i q) -> p i q", i=2),
                        in0=pt[:].rearrange("p (i q) -> p i q", i=2),
                        in1=mkt[:, ko % 8, None, :].to_broadcast([P, 2, QQ]),
                    )
                    self.ptq[ko] = pt
                    if do_drain and ko % 2 == 1:
                        drain(1)
                self.next_ko = ko_end

            def finish(self):
                qq, hp = self.qq, self.hp
                for ko in range(KO - SKEW, KO):
                    self.emit_pv(ko)
                qsl = slice(qq * QQ, (qq + 1) * QQ)
                # pv0: rows 0-63 rowsum A (replicated), 64-127 head A data;
                # pv1: rows 0-63 head B data, 64-127 rowsum B.
                # Rowsum reciprocal as exp(-ln(x)) on ScalarE: Ln and Exp
                # share an activation table (no table-switch cost), DVE
                # divide is ~3.4us per call here, and ScalarE has slack. The
                # rowsum halves land partition-aligned by the Vi layout; head
                # evacuations are single-input copies so they may cross
                # partition bases (TensorTensor may not).
                lnt = lnp.tile([P, QQ], f32, tag="ln", name=f"ln{qq}_{hp}")
                rt = rtp.tile([P, QQ], bf16, tag="rt", name=f"rt{qq}_{hp}")
                nc.vector.tensor_copy(
                    out=headiT_s[hp][0:64, qsl], in_=self.pv[0][64:128, :]
                )
                nc.scalar.activation(
                    out=lnt[0:64, :], in_=self.pv[0][0:64, :], func=Ln, scale=1.0
                )
                nc.vector.tensor_copy(
                    out=headiT_s[hp][64:128, qsl], in_=self.pv[1][0:64, :]
                )
                nc.scalar.activation(
                    out=lnt[64:128, :], in_=self.pv[1][64:128, :], func=Ln, scale=1.0
                )
                nc.scalar.activation(out=rt[:], in_=lnt[:], func=Exp, scale=-1.0)
                nc.vector.tensor_mul(
                    out=headiT_s[hp][:, qsl],
                    in0=headiT_s[hp][:, qsl],
                    in1=rt[:],
                )

        # ---------- projection era: V + K stripes, pass (0,0) streamed ----
        p00 = None
        with ExitStack() as ph:
            wvp = ph.enter_context(tc.tile_pool(name="wvres", bufs=1))
            wkp = ph.enter_context(tc.tile_pool(name="wkp", bufs=1))
            vsp = ph.enter_context(tc.tile_pool(name="vstripe", bufs=3))
            ksp = ph.enter_context(tc.tile_pool(name="kstripe", bufs=2))

            vtiles = {}
            ktiles = {}

            def load_v(hs):
                vs = vsp.tile([P, DO, 256], bf16, tag="v", name=f"vT{hs}")
                nc.sync.dma_start(vs[:], vT_t[hs])
                vtiles[hs] = vs

            def load_k(s):
                ks = ksp.tile([P, DO, 512], bf16, tag="k", name=f"kT{s}")
                nc.sync.dma_start(ks[:], kT_t[s])
                ktiles[s] = ks

            # DMA issue order = arrival order; feed the V/K critical path
            # first so PE starts ~35us earlier than loading q/mask up front.
            wv_s = wvp.tile([P, DO, EH], bf16)
            nc.sync.dma_start(wv_s[:], wv_t[:])
            load_v(0)
            wk_s = wkp.tile([P, EO, DO, P], bf16)
            nc.sync.dma_start(wk_s[:], wk_t[:])
            load_k(0)
            load_v(1)
            load_k(1)
            load_v(2)
            nc.sync.dma_start(wq_s[:, 0], wq_t[0])
            # ones blocks for the PV rowsum trick
            vi_all = Vi_s[:].rearrange("p k (pr c) -> p k pr c", c=VB)
            nc.vector.memset(vi_all[:, :, :, 0:64], 1.0)
            nc.vector.memset(vi_all[:, :, :, 192:256], 1.0)

            for s in range(4):
                for half in range(2):
                    hs = 2 * s + half
                    vs = vtiles.pop(hs)
                    for t in range(2):
                        ko = hs * 2 + t
                        pt = gps.tile([P, 512], f32, tag="gps", name=f"psv{ko}")
                        for do in range(DO):
                            nc.tensor.matmul(
                                pt[:],
                                lhsT=vs[:, do, t * P : (t + 1) * P],
                                rhs=wv_s[:, do],
                                start=(do == 0),
                                stop=(do == DO - 1),
                            )
                        vi_ko = Vi_s[:, ko].rearrange("p (pr c) -> p pr c", c=VB)
                        nc.vector.tensor_copy(
                            out=vi_ko[:, :, 64:192],
                            in_=pt[:].rearrange("p (pr e) -> p pr e", e=128),
                        )
                    if hs + 3 <= 7:
                        load_v(hs + 3)
                ks = ktiles.pop(s)
                for eo in range(EO):
                    pt = gps.tile([P, 512], f32, tag="gps", name=f"psk{s}_{eo}")
                    for do in range(DO):
                        nc.tensor.matmul(
                            pt[:],
                            lhsT=wk_s[:, eo, do],
                            rhs=ks[:, do],
                            start=(do == 0),
                            stop=(do == DO - 1),
                        )
                    nc.vector.tensor_copy(
                        out=KiT_s[:, eo, s * 512 : (s + 1) * 512], in_=pt[:]
                    )
                # prefetches + deferred loads, then stream pass (0,0)
                if s == 0:
                    load_k(2)
                    load_qstripe(0)
                elif s == 1:
                    load_k(3)
                    load_mask(0)
                    q_proj(0, 0)
                    p00 = AttnPass(0, 0)
                elif s == 2:
                    for eo in range(1, EO):
                        nc.sync.dma_start(wq_s[:, eo], wq_t[eo])
                    nc.sync.dma_start(wo_s[:], wo_t[:])
                    p00.steps(4, do_drain=False)
                elif s == 3:
                    p00.steps(8, do_drain=False)

        def out_proj(qq):
            for t in range(QQ // P):  # 4 q-tiles of 128
                qt = qq * (QQ // P) + t
                for dc in range(2):
                    pt = gps.tile([P, 512], f32, tag="gps", name=f"pso{qt}_{dc}")
                    # split the accumulation so the eo<3 matmuls don't
                    # inherit the last head's normalize dependency (waits
                    # hoist to the group head); lets out-proj overlap the
                    # final pass's normalize chain
                    for eo in range(EO):
                        nc.tensor.matmul(
                            pt[:],
                            lhsT=headiT_s[eo][:, qt * P : (qt + 1) * P],
                            rhs=wo_s[:, eo, dc * 512 : (dc + 1) * 512],
                            start=(eo == 0),
                            stop=(eo >= EO - 2),
                            skip_group_check=(eo == EO - 1),
                        )
                    ot = nrm.tile([P, 512], bf16, tag="ot", name=f"o{qt}_{dc}")
                    nc.vector.tensor_copy(out=ot[:], in_=pt[:])
                    nc.sync.dma_start(out_t[qt, dc], ot[:])
                    yield

        def out_proj_part(qq, eos, dst):
            for t in range(QQ // P):
                qt = qq * (QQ // P) + t
                for dc in range(2):
                    pt = gps.tile(
                        [P, 512], f32, tag="gps", name=f"psp{qt}_{dc}_{eos[0]}"
                    )
                    for i, eo in enumerate(eos):
                        nc.tensor.matmul(
                            pt[:],
                            lhsT=headiT_s[eo][:, qt * P : (qt + 1) * P],
                            rhs=wo_s[:, eo, dc * 512 : (dc + 1) * 512],
                            start=(i == 0),
                            stop=(i == len(eos) - 1),
                        )
                    ot = nrm.tile(
                        [P, 512], bf16, tag="ot", name=f"op{qt}_{dc}_{eos[0]}"
                    )
                    nc.vector.tensor_copy(out=ot[:], in_=pt[:])
                    if dst is None:
                        nc.sync.dma_start(out_t[qt, dc], ot[:])
                    else:
                        nc.sync.dma_start(dst[t, dc], ot[:])
                    yield

        for eo in range(1, EO):
            filler.append(lambda eo=eo: q_proj(0, eo, (0,)))
            filler.append(lambda eo=eo: q_proj(0, eo, (1,)))

        def queue_setup(qq):
            filler.append(lambda: (load_mask(qq), load_qstripe(qq)))
            for eo in range(EO):
                filler.append(lambda eo=eo: q_proj(qq, eo, (0,)))
                filler.append(lambda eo=eo: q_proj(qq, eo, (1,)))

        # ---------- attention passes (software-pipelined across passes) ---
        def notify_finished(p):
            if p.qq == NQQ - 1:
                if p.hp == 1:
                    filler.append(out_proj_part(p.qq, (0, 1), out2_t))
                elif p.hp == EO - 1:
                    filler.append(out_proj_part(p.qq, (2, 3), None))
            elif p.hp == EO - 1:
                filler.append(out_proj(p.qq))

        prev = None
        for qq in range(NQQ):
            if qq + 1 < NQQ:
                queue_setup(qq + 1)
            for hp in range(EO):
                if qq == 0 and hp == 0:
                    p00.steps(KO, do_drain=False)
                    prev = p00
                    continue
                while (qq, hp) not in q_done or qq not in mask_tiles:
                    drain(1)
                ap = AttnPass(qq, hp)
                ap.steps(SKEW, do_drain=False)
                if prev is not None:
                    prev.finish()
                    notify_finished(prev)
                    # stuff the pass boundary with filler so PE covers the
                    # DVE pv-drain (WAR on the recycled pv psum banks)
                    drain(3)
                ap.steps(KO)
                prev = ap
        prev.finish()
        notify_finished(prev)
        while filler:
            drain(1)

    nc.compile()
    return nc


def get_program():
    if "nc" not in _prog_cache:
        _prog_cache["nc"] = _build_program()
    return _prog_cache["nc"]


def make_in_maps(K, Q, V, mask, WQ, WK, WV, WO_w, WO_b):
    bf = ml_dtypes.bfloat16
    K = np.asarray(K, dtype=np.float32)
    Q = np.asarray(Q, dtype=np.float32)
    V = np.asarray(V, dtype=np.float32)
    mask = np.asarray(mask)
    WQ = np.asarray(WQ, dtype=np.float32)
    WK = np.asarray(WK, dtype=np.float32)
    WV = np.asarray(WV, dtype=np.float32)
    woT = np.asarray(WO_w, dtype=np.float32).T  # (E, DMODEL)

    def tile_dq(xT, stripes, width):
        # (D, L) -> (stripes, P, DO, width), contiguous per partition
        return np.ascontiguousarray(
            xT.reshape(DO, P, stripes, width).transpose(2, 1, 0, 3)
        )

    qT_b = [tile_dq(Q[n].T.astype(bf), NQQ, QQ) for n in range(N)]
    kT_b = [tile_dq(K[n].T.astype(bf), 4, 512) for n in range(N)]
    vT_b = [tile_dq(V[n].T.astype(bf), 8, 256) for n in range(N)]
    maskT_b = [
        np.ascontiguousarray(
            mask[n, 0]
            .T.astype(bf)
            .reshape(2, KO // 2, P, NQQ, QQ)
            .transpose(3, 0, 2, 1, 4)
        )
        for n in range(N)
    ]

    in_maps = []
    for c in range(NCORES):
        n, hh = c // 2, c % 2
        hs = slice(hh * HPC, (hh + 1) * HPC)
        # head-concat weight slices: (HPC, D, DK) -> (D, HPC*DK)
        wq_h = np.ascontiguousarray(WQ[hs].transpose(1, 0, 2).reshape(DMODEL, EH))
        wk_h = np.ascontiguousarray(WK[hs].transpose(1, 0, 2).reshape(DMODEL, EH))
        wv_h = WV[hs].transpose(1, 0, 2).reshape(DMODEL, EH)
        wo_h = woT[hh * EH : (hh + 1) * EH, :]
        in_maps.append(
            {
                "qT": qT_b[n],
                "kT": kT_b[n],
                "vT": vT_b[n],
                "maskT": maskT_b[n],
                "wq": np.ascontiguousarray(
                    wq_h.reshape(DO, P, EO, P).transpose(2, 1, 0, 3).astype(bf)
                ),
                "wk": np.ascontiguousarray(
                    wk_h.reshape(DO, P, EO, P).transpose(1, 2, 0, 3).astype(bf)
                ),
                "wv": np.ascontiguousarray(
                    wv_h.reshape(DO, P, EH).transpose(1, 0, 2).astype(bf)
                ),
                "wo": np.ascontiguousarray(
                    wo_h.reshape(EO, P, DMODEL).transpose(1, 0, 2).astype(bf)
                ),
            }
        )
    return in_maps


def kernel(K, Q, V, mask, WQ, WK, WV, WO_w, WO_b):
    from concourse import bass_utils

    nc = get_program()
    in_maps = make_in_maps(K, Q, V, mask, WQ, WK, WV, WO_w, WO_b)
    res = bass_utils.run_bass_kernel_spmd(
        nc, in_maps, core_ids=list(range(NCORES)), trace=False
    )
    bias = np.asarray(WO_b, dtype=np.float32).reshape(1, DMODEL)
    out = np.empty((N, QLEN, DMODEL), dtype=np.float32)
    for n in range(N):
        o0 = res.results[2 * n]["out"].astype(np.float32)
        o1 = res.results[2 * n + 1]["out"].astype(np.float32)
        o0[QT - 4 :] += res.results[2 * n]["out2"].astype(np.float32)
        o1[QT - 4 :] += res.results[2 * n + 1]["out2"].astype(np.float32)
        full = (o0 + o1).transpose(0, 2, 1, 3).reshape(QLEN, DMODEL)
        out[n] = full + bias
    return out
